# revision 25
# baseline (speedup 1.0000x reference)
"""Trainium2 Bass kernel for MultiHeadSelfAttention with low-rank score
projection (LSR), causal softmax — 8-core SPMD.

Sharding: core i computes batch b = i//2 and head-group g = i%2 (8 of the
16 heads).  Each core produces a partial transposed output
out.T = Wo_slice @ y.T of shape [1024, 2048]; the host sums the two
head-group partials per batch, transposes, and adds bo.

On-core layout is feature-major ("transposed"): q.T/k.T are [dims, tokens]
so the scores matmul S.T = k_lr @ q_lr.T puts k-tokens on partitions,
letting P.T = exp(S.T) feed the P@V matmul directly (contraction over
k-tokens).  v is token-major with a ones-column appended so the softmax
denominators fall out of the same matmul (row 64 of Y.T in PSUM).
All matmuls run in float32r (~1e-4 matmul rel-err, full PE rate at
moving-dim >= 256).
"""

import sys

if "/opt/trn_rl_repo" not in sys.path:
    sys.path.insert(0, "/opt/trn_rl_repo")

import math

import numpy as np

import concourse.bass as bass
import concourse.tile as tile
from concourse import bacc, mybir

D_MODEL = 1024
N_HEADS = 16
D_HEAD = 64
LSR_RANK = 32
B, T = 4, 2048
N_CORES = 8
HPC = 8                      # heads per core
DSL = HPC * D_HEAD           # 512: per-core slice of d_model
NCHUNK = T // 512            # 4 token chunks of 512
NKT = D_MODEL // 128         # 8 k-tiles over d_model
SCALE = 1.0 / math.sqrt(LSR_RANK)

F32 = mybir.dt.float32
F32R = mybir.dt.float32r


def build_program():
    nc = bacc.Bacc("TRN2", target_bir_lowering=False, debug=False)

    d = {}
    d["xT"] = nc.dram_tensor("xT", [D_MODEL, T], F32R, kind="ExternalInput").ap()
    d["WqT"] = nc.dram_tensor("WqT", [D_MODEL, DSL], F32R, kind="ExternalInput").ap()
    d["WkT"] = nc.dram_tensor("WkT", [D_MODEL, DSL], F32R, kind="ExternalInput").ap()
    d["WvT"] = nc.dram_tensor("WvT", [D_MODEL, DSL], F32R, kind="ExternalInput").ap()
    d["WoT"] = nc.dram_tensor("WoT", [DSL, D_MODEL], F32R, kind="ExternalInput").ap()
    d["lsr_q"] = nc.dram_tensor("lsr_q", [4, 128, 128], F32R, kind="ExternalInput").ap()
    d["lsr_k"] = nc.dram_tensor("lsr_k", [4, 128, 128], F32R, kind="ExternalInput").ap()
    d["bq_s"] = nc.dram_tensor("bq_s", [128, 4], F32, kind="ExternalInput").ap()
    d["bk_s"] = nc.dram_tensor("bk_s", [128, 4], F32, kind="ExternalInput").ap()
    d["bv_b"] = nc.dram_tensor("bv_b", [128, DSL], F32, kind="ExternalInput").ap()
    d["tri"] = nc.dram_tensor("tri", [128, 128], F32, kind="ExternalInput").ap()
    d["outT"] = nc.dram_tensor("outT", [D_MODEL, T], F32, kind="ExternalOutput").ap()

    with tile.TileContext(nc) as tc, \
         nc.allow_low_precision(reason="fp32r attention; verified vs fp32 ref"):
        _build(nc, tc, d)
    nc.compile()
    return nc


def _build(nc, tc, d):
    import contextlib

    ctx = contextlib.ExitStack()
    consts = ctx.enter_context(tc.tile_pool(name="consts", bufs=1))
    xpool = ctx.enter_context(tc.tile_pool(name="xpool", bufs=1))
    qkst = ctx.enter_context(tc.tile_pool(name="qkst", bufs=2))
    ptpool = ctx.enter_context(tc.tile_pool(name="ptpool", bufs=2))
    misc = ctx.enter_context(tc.tile_pool(name="misc", bufs=2))
    ps_mm = ctx.enter_context(tc.tile_pool(name="ps_mm", bufs=2, space="PSUM"))
    ps_lsr = ctx.enter_context(tc.tile_pool(name="ps_lsr", bufs=1, space="PSUM"))
    ps_st = ctx.enter_context(tc.tile_pool(name="ps_st", bufs=2, space="PSUM"))
    ps_yt = ctx.enter_context(tc.tile_pool(name="ps_yt", bufs=2, space="PSUM"))
    ps_rb = ctx.enter_context(tc.tile_pool(name="ps_rb", bufs=1, space="PSUM"))

    # ---- constants / weights ----
    wq = [consts.tile([128, DSL], F32R, tag=f"wq{k}", name=f"wq{k}") for k in range(NKT)]
    wk = [consts.tile([128, DSL], F32R, tag=f"wk{k}", name=f"wk{k}") for k in range(NKT)]
    wv = [consts.tile([128, DSL], F32R, tag=f"wv{k}", name=f"wv{k}") for k in range(NKT)]
    for k in range(NKT):
        nc.sync.dma_start(wq[k][:], d["WqT"][128 * k:128 * (k + 1), :])
    for k in range(NKT):
        nc.sync.dma_start(wk[k][:], d["WkT"][128 * k:128 * (k + 1), :])
    for k in range(NKT):
        nc.sync.dma_start(wv[k][:], d["WvT"][128 * k:128 * (k + 1), :])
    wo = [consts.tile([128, D_MODEL], F32R, tag=f"wo{k}", name=f"wo{k}") for k in range(4)]
    for k in range(4):
        nc.sync.dma_start(wo[k][:], d["WoT"][128 * k:128 * (k + 1), :])
    lsrq = [consts.tile([128, 128], F32R, tag=f"lsrq{m}", name=f"lsrq{m}") for m in range(4)]
    lsrk = [consts.tile([128, 128], F32R, tag=f"lsrk{m}", name=f"lsrk{m}") for m in range(4)]
    for m in range(4):
        nc.sync.dma_start(lsrq[m][:], d["lsr_q"][m])
        nc.sync.dma_start(lsrk[m][:], d["lsr_k"][m])
    bq = consts.tile([128, 4], F32, tag="bq")
    bk = consts.tile([128, 4], F32, tag="bk")
    bvb = consts.tile([128, DSL], F32, tag="bvb")
    tri = consts.tile([128, 128], F32, tag="tri")
    nc.sync.dma_start(bq[:], d["bq_s"][:])
    nc.sync.dma_start(bk[:], d["bk_s"][:])
    nc.sync.dma_start(bvb[:], d["bv_b"][:])
    nc.sync.dma_start(tri[:], d["tri"][:])
    ones_sel = consts.tile([128, 64], F32R, tag="ones_sel")
    nc.vector.memset(ones_sel[64:65, :].bitcast(F32), 1.0)

    # persistent intermediates
    qlr = [consts.tile([128, T], F32R, tag=f"qlr{t}", name=f"qlr{t}") for t in range(2)]
    klr = [consts.tile([128, T], F32R, tag=f"klr{t}", name=f"klr{t}") for t in range(2)]
    ynorm = [consts.tile([128, T], F32R, tag=f"yn{t}", name=f"yn{t}") for t in range(4)]
    vaug = {}                    # token-tile j -> [128, 8*65] tile

    for c in range(NCHUNK):
        cs = slice(512 * c, 512 * (c + 1))
        # ---- x chunk load ----
        xt = [xpool.tile([128, 512], F32R, tag=f"x{k}", name=f"x{k}") for k in range(NKT)]
        for k in range(NKT):
            nc.sync.dma_start(xt[k][:], d["xT"][128 * k:128 * (k + 1), cs])

        # ---- q.T / k.T projections + LSR ----
        for which, w, bias, lsrw, lrout in (
            ("q", wq, bq, lsrq, qlr), ("k", wk, bk, lsrk, klr),
        ):
            for t in range(2):
                pl = ps_lsr.tile([128, 512], F32, tag="lsr")
                for mh in range(2):
                    m = 2 * t + mh
                    pp = ps_mm.tile([128, 512], F32, tag="mm")
                    for k in range(NKT):
                        nc.tensor.matmul(
                            pp[:], w[k][:, 128 * m:128 * (m + 1)], xt[k][:],
                            start=(k == 0), stop=(k == NKT - 1))
                    qt = qkst.tile([128, 512], F32R, tag=f"{which}t")
                    nc.vector.tensor_scalar_add(qt[:], pp[:], bias[:, m:m + 1])
                    nc.tensor.matmul(
                        pl[:], lsrw[m][:], qt[:],
                        start=(mh == 0), stop=(mh == 1))
                nc.vector.tensor_copy(lrout[t][:, cs], pl[:])

        # ---- v (token-major) + ones column ----
        for tt in range(4):
            j = 4 * c + tt       # global token tile
            pp = ps_mm.tile([128, 512], F32, tag="mm")
            for k in range(NKT):
                nc.tensor.matmul(
                    pp[:], xt[k][:, 128 * tt:128 * (tt + 1)], wv[k][:],
                    start=(k == 0), stop=(k == NKT - 1))
            va = consts.tile([128, 8 * 65], F32R, tag=f"vaug{j}")
            vaug[j] = va
            va3 = va[:].rearrange("p (h e) -> p h e", e=65)
            nc.vector.tensor_tensor(
                out=va3[:, :, 0:64],
                in0=pp[:].rearrange("p (h e) -> p h e", e=64),
                in1=bvb[:].rearrange("p (h e) -> p h e", e=64),
                op=mybir.AluOpType.add)
            nc.vector.memset(va3[:, :, 64:65].bitcast(F32), 1.0)

        # ---- attention for chunk c: head pairs, score MMs emitted
        # adjacently at different tile_position rows so the PE runs them
        # concurrently (row-group tiling) ----
        jmax = 4 * c + 3
        for hp in range(4):
            heads = (2 * hp, 2 * hp + 1)
            yts = {}
            for h in heads:
                yts[h] = ps_yt.tile([65, 512], F32, tag="yt", name=f"yt{h}")
            for j in range(jmax + 1):
                pts = {}
                for h in heads:
                    kt, kb = h // 4, 32 * (h % 4)
                    st = ps_st.tile([128, 512], F32, tag="st", name=f"st{h}")
                    nc.tensor.matmul(
                        st[:], klr[kt][kb:kb + 32, 128 * j:128 * (j + 1)],
                        qlr[kt][kb:kb + 32, cs], start=True, stop=True,
                        tile_position=(kb, 0))
                    pts[h] = (st, ptpool.tile([128, 512], F32R, tag="pt",
                                              name=f"pt{h}"))
                r = j - 4 * c
                for h in heads:
                    st, pt = pts[h]
                    nc.scalar.activation(
                        pt[:], st[:], mybir.ActivationFunctionType.Exp,
                        scale=SCALE)
                    if r > 0:
                        nc.vector.memset(pt[:, 0:128 * r].bitcast(F32), 0.0)
                    if r >= 0:
                        nc.vector.tensor_mul(
                            pt[:, 128 * r:128 * (r + 1)],
                            pt[:, 128 * r:128 * (r + 1)], tri[:])
                for h in heads:
                    nc.tensor.matmul(
                        yts[h][:], vaug[j][:, 65 * h:65 * h + 65],
                        pts[h][1][:], start=(j == 0), stop=(j == jmax))
            for h in heads:
                yt = yts[h]
                rec = misc.tile([128, 512], F32R, tag="rec", name=f"rec{h}")
                nc.vector.reciprocal(rec[64:65, :], yt[64:65, :])
                rb = ps_rb.tile([64, 512], F32, tag="rb", name=f"rb{h}")
                nc.tensor.matmul(rb[:], ones_sel[64:65, :], rec[64:65, :],
                                 start=True, stop=True)
                rbs = misc.tile([64, 512], F32, tag="rbs", name=f"rbs{h}")
                nc.vector.tensor_copy(rbs[:], rb[:])
                nc.vector.tensor_mul(
                    ynorm[h // 2][64 * (h % 2):64 * (h % 2) + 64, cs],
                    yt[0:64, :], rbs[:])

        # ---- output projection for chunk c ----
        for m in range(8):
            po = ps_mm.tile([128, 512], F32, tag="mm")
            for k in range(4):
                nc.tensor.matmul(
                    po[:], wo[k][:, 128 * m:128 * (m + 1)], ynorm[k][:, cs],
                    start=(k == 0), stop=(k == 3))
            oc = misc.tile([128, 512], F32, tag="oc")
            nc.vector.tensor_copy(oc[:], po[:])
            nc.sync.dma_start(d["outT"][128 * m:128 * (m + 1), cs], oc[:])

    ctx.close()


_PROGRAM_CACHE = {}


def _get_program():
    if "nc" not in _PROGRAM_CACHE:
        _PROGRAM_CACHE["nc"] = build_program()
    return _PROGRAM_CACHE["nc"]


def make_in_maps(x, Wq, bq, Wk, bk, Wv, bv, Wo, bo, Wq_lsr, Wk_lsr):
    """Per-core input dicts. Core i: batch i//2, head-group i%2."""
    x = np.asarray(x, dtype=np.float32)
    f32 = lambda a: np.ascontiguousarray(np.asarray(a, dtype=np.float32))
    WqT, WkT, WvT, WoT = (f32(W.T) for W in (Wq, Wk, Wv, Wo))
    tri = np.triu(np.ones((128, 128), dtype=np.float32))
    in_maps = []
    for core in range(N_CORES):
        b, g = core // 2, core % 2
        hsl = slice(DSL * g, DSL * (g + 1))
        hs = [g * HPC + h for h in range(HPC)]
        lsr_q = np.zeros((4, 128, 128), dtype=np.float32)
        lsr_k = np.zeros((4, 128, 128), dtype=np.float32)
        for m in range(4):
            cb = 64 * (m % 2)
            lsr_q[m, 0:64, cb:cb + 32] = Wq_lsr[hs[2 * m]]
            lsr_q[m, 64:128, cb + 32:cb + 64] = Wq_lsr[hs[2 * m + 1]]
            lsr_k[m, 0:64, cb:cb + 32] = Wk_lsr[hs[2 * m]]
            lsr_k[m, 64:128, cb + 32:cb + 64] = Wk_lsr[hs[2 * m + 1]]
        in_maps.append({
            "xT": f32(x[b].T),
            "WqT": f32(WqT[:, hsl]),
            "WkT": f32(WkT[:, hsl]),
            "WvT": f32(WvT[:, hsl]),
            "WoT": f32(WoT[hsl, :]),
            "lsr_q": lsr_q,
            "lsr_k": lsr_k,
            "bq_s": f32(np.asarray(bq)[hsl].reshape(4, 128).T),
            "bk_s": f32(np.asarray(bk)[hsl].reshape(4, 128).T),
            "bv_b": f32(np.tile(np.asarray(bv)[hsl], (128, 1))),
            "tri": tri,
        })
    return in_maps


def run(inputs, trace=False):
    """Run on 8 cores; returns (full_output [B,T,D], exec_time_ns or None)."""
    from concourse.bass_utils import run_bass_kernel_spmd

    try:
        from antenv import axon_hooks  # ensure NTFF hook shim is importable
        axon_hooks.ensure_registered()
    except Exception:
        pass

    nc = _get_program()
    in_maps = make_in_maps(**inputs)
    res = run_bass_kernel_spmd(nc, in_maps, core_ids=list(range(N_CORES)),
                               trace=trace)
    bo = np.asarray(inputs["bo"], dtype=np.float32)
    out = np.empty((B, T, D_MODEL), dtype=np.float32)
    for b in range(B):
        acc = res.results[2 * b]["outT"] + res.results[2 * b + 1]["outT"]
        out[b] = acc.T + bo
    return out, res.exec_time_ns


def kernel(**inputs) -> np.ndarray:
    out, _ = run(inputs, trace=False)
    return out
